# revision 1
# baseline (speedup 1.0000x reference)
"""Bass/Trainium2 kernel for nn_BERT_TUCKER (BERT + TuckER pair scoring).

z[b,k,t,r] = sum_{a,j} head[b,k,a] * Wv[a,r,j] * tail[b,t,j],
Wv = W.reshape(808, 50, 808)  (130.6 MB fp32; read-once => memory roofline).

Sharding: slice Wv's first (head-contraction) dim a=808 into 8 slices of
101 across cores; each core holds Wc = Wv[a_slice] transposed to [j,r,a]
in bf16 (8.2 MB/core) and computes
  m1: U[a, r, (b,t)] = sum_j Wc[j,r,a] * tailT[j, bt]   (contract j, 7 chunks)
  m2: z[k, (r,t)] per b = heads[a, k].T @ U[a, r, t]    (contract a, 1 chunk)
Partial z (over the a-slice) is summed on host; batchnorm + R projection
are affine in z, so they're applied after the sum (exact).

bf16 halves the HBM traffic (memory regime) and runs the PE at full rate
with no f32r free-dim>=256 constraint; accumulation stays fp32 in PSUM.
Mention/entity pooling (~0.5 GFLOP) is done on host into ent.
"""

import numpy as np
from ml_dtypes import bfloat16

B, S, H = 16, 512, 768
TS, IS = 20, 20
D = H + TS + IS          # 808
M = 36
E = 12
R_NUM = 97
D2 = 50
EPS = 1e-5

NCORES = 8
ASL = D // NCORES        # 101 per-core a-slice
JP = 896                 # j padded to 7*128
NJC = 7                  # j chunks of 128
NBK = B * E              # 192
RBLOCKS = (4, 6, 8, 10, 12, 6, 4)  # r blocks (even sizes; ramped so each
RB_MAX = max(RBLOCKS)            # block's W DMA hides under prior compute)

_CACHE = {}


def _pool_entities(encoder_hidden, entity_type, entity_id, mention_id,
                   entity2mention_table, type_emb, id_emb):
    """Steps 1-3 of the reference (embedding concat + mention/entity
    pooling) on host. Returns ent [B, E, D] fp32."""
    enc = np.concatenate(
        [encoder_hidden, type_emb[entity_type], id_emb[entity_id]], axis=-1
    ).astype(np.float32)                                   # [B,S,D]
    cls = np.concatenate(
        [encoder_hidden[:, 0, :], np.zeros((B, TS + IS), np.float32)], axis=-1
    )                                                      # [B,D]

    sel = (np.arange(1, M + 1, dtype=mention_id.dtype)[None, :, None]
           == mention_id[:, None, :]).astype(np.float32)   # [B,M,S]
    cnt = sel.sum(axis=-1, keepdims=True)
    sel = np.where(cnt > 0, sel / np.maximum(cnt, 1), sel)
    x = np.matmul(sel, enc)                                # [B,M,D]
    x = np.concatenate([cls[:, None, :], x], axis=1)       # [B,M+1,D]

    tbl = entity2mention_table.astype(np.float32).copy()
    tbl[:, 0, 0] = 1.0
    mcnt = tbl.sum(axis=-1, keepdims=True)
    tbl = np.where(mcnt > 0, tbl / np.maximum(mcnt, 1), tbl)
    return np.matmul(tbl, x)[:, 1:, :]                     # [B,E,D]


def _w_fingerprint(W):
    s = np.ascontiguousarray(W[::7, ::101, ::97])
    return (W.shape, str(W.dtype), s.tobytes())


def _prepare_w(W):
    """Per-core W slices transposed to [j, r, a] bf16, j chunked and
    partition-major: [128, 7, 50, 101]. Cached (W is static across calls)."""
    key = _w_fingerprint(W)
    hit = _CACHE.get("wprep")
    if hit is not None and hit[0] == key:
        return hit[1]
    Wv = W.reshape(D, D2, D)                               # [a, r, j] view
    cores = []
    for c in range(NCORES):
        a0 = c * ASL
        wc = Wv[a0:a0 + ASL].astype(bfloat16)              # [101, 50, 808]
        wt = np.zeros((JP, D2, ASL), bfloat16)
        wt[:D] = wc.transpose(2, 1, 0)                     # [808, 50, 101]
        # partition-major [128, jc, r, a] so one DMA covers all j chunks
        cores.append(np.ascontiguousarray(
            wt.reshape(NJC, 128, D2, ASL).transpose(1, 0, 2, 3)))
    _CACHE["wprep"] = (key, cores)
    return cores


def _host_prepare(encoder_hidden, entity_type, entity_id, mention_id,
                  entity2mention_table, type_emb, id_emb, W):
    ent = _pool_entities(encoder_hidden, entity_type, entity_id, mention_id,
                         entity2mention_table, type_emb, id_emb)
    ent_flat = ent.reshape(NBK, D)                         # [(b,t), D]

    entT = np.zeros((JP, NBK), np.float32)
    entT[:D] = ent_flat.T
    # partition-major [128, jc, bt] so the single DMA is shape-congruent
    entT_dev = np.ascontiguousarray(
        entT.astype(bfloat16).reshape(NJC, 128, NBK).transpose(1, 0, 2))

    w_cores = _prepare_w(W)
    in_maps = []
    for c in range(NCORES):
        a0 = c * ASL
        in_maps.append({
            "WcT": w_cores[c],
            "entT": entT_dev,
            "entH": np.ascontiguousarray(
                ent_flat[:, a0:a0 + ASL].T).astype(bfloat16),  # [101, 192]
        })
    return in_maps, ent


def _build_bass():
    import concourse.bacc as bacc
    import concourse.mybir as mybir
    import concourse.tile as tile

    f32 = mybir.dt.float32
    bf16 = mybir.dt.bfloat16

    nc = bacc.Bacc("TRN2", target_bir_lowering=False, debug=False)
    JC6 = D - (NJC - 1) * 128                  # 40 real rows in last j chunk
    WcT_d = nc.dram_tensor("WcT", (128, NJC, D2, ASL), bf16,
                           kind="ExternalInput")
    entT_d = nc.dram_tensor("entT", (128, NJC, NBK), bf16,
                            kind="ExternalInput")
    entH_d = nc.dram_tensor("entH", (ASL, NBK), bf16, kind="ExternalInput")
    out_z = nc.dram_tensor("out_z", (E, B, D2, E), f32, kind="ExternalOutput")
    dbg_d = nc.dram_tensor("dbg", (1, 64), f32, kind="ExternalOutput")

    with tile.TileContext(nc) as tc:
        with (
            tc.tile_pool(name="const", bufs=1) as cpool,
            tc.tile_pool(name="wpool", bufs=4) as wpool,
            tc.tile_pool(name="zpool", bufs=3) as zpool,
            tc.tile_pool(name="ps_u", bufs=4, space="PSUM") as ps_u,
            tc.tile_pool(name="ps_z", bufs=4, space="PSUM") as ps_z,
        ):
            entT_sb = cpool.tile([128, NJC, NBK], bf16, tag="entT")
            nc.gpsimd.dma_start(entT_sb[:].rearrange("p a b -> p (a b)"),
                                entT_d[:].rearrange("p a b -> p (a b)"))
            entH_sb = cpool.tile([ASL, NBK], bf16, tag="entH")
            nc.scalar.dma_start(entH_sb[:], entH_d[:])

            # PE warm-up during the initial DMA head (HAM un-throttle).
            wu = cpool.tile([128, 128], bf16, tag="warm")
            nc.vector.memset(wu[:], 0.0)
            wps = ps_z.tile([128, 64], f32, tag="zt")
            NWARM = 34
            for i in range(NWARM):
                nc.tensor.matmul(wps[:], wu[:], wu[:, 0:64],
                                 start=(i == 0), stop=(i == NWARM - 1))

            U_sb = cpool.tile([ASL, D2, NBK], bf16, tag="U")

            r0 = 0
            for rb in RBLOCKS:
                w_t = wpool.tile([128, NJC, RB_MAX, ASL], bf16, tag="w")
                # two DMAs per block: j chunks 0-5 merged, then the 40 real
                # rows of chunk 6 (its zero-padding rows are never read —
                # entT's zero rows kill any garbage in partitions 40+).
                nc.sync.dma_start(w_t[:, 0:NJC - 1, 0:rb, :],
                                  WcT_d[:, 0:NJC - 1, r0:r0 + rb, :])
                nc.gpsimd.dma_start(w_t[0:JC6, NJC - 1, 0:rb, :],
                                    WcT_d[0:JC6, NJC - 1, r0:r0 + rb, :])
                # m1: U[a, r, bt] += Wc[j,r,a].T @ tailT[j, bt], 2 r per bank
                for rp in range(rb // 2):
                    pu = ps_u.tile([ASL, 2, NBK], f32, tag="pu")
                    for half in range(2):
                        rl = rp * 2 + half
                        for jc in range(NJC):
                            # last j chunk has only 40 real rows; contract
                            # just those (w_t rows 40+ are never written)
                            np_ = 128 if jc < NJC - 1 else JC6
                            nc.tensor.matmul(
                                pu[:, half, :],
                                w_t[0:np_, jc, rl, :],
                                entT_sb[0:np_, jc, :],
                                start=(jc == 0), stop=(jc == NJC - 1),
                            )
                    if rp % 2 == 0:
                        nc.vector.tensor_copy(
                            U_sb[:, r0 + rp * 2:r0 + rp * 2 + 2, :], pu[:])
                    else:
                        nc.scalar.copy(
                            U_sb[:, r0 + rp * 2:r0 + rp * 2 + 2, :], pu[:])
                # m2: z[k, r, t] per b for this r block (contract a-slice).
                # Per-block z tile (pool slot) so a block's output DMA never
                # WAR-serializes against the next block's copies; z copies +
                # most z DMAs live on ACT so their sem-waits never block
                # SP's W-prefetch SEQ.
                last = (r0 + rb == D2)
                z_sb = zpool.tile([E, B, RB_MAX, E], f32, tag="z")
                dma_engs = (nc.scalar, nc.gpsimd, nc.sync)
                # group size: as many b per PSUM bank as fit for this rb
                # (the final block keeps 3 so its tail chain stays small)
                g = 3 if last else min(B, 512 // (rb * E))
                for bg in range((B + g - 1) // g):
                    b0 = bg * g
                    nb = min(g, B - b0)
                    zt = ps_z.tile([E, g, rb, E], f32, tag="zt")
                    for i in range(nb):
                        b = b0 + i
                        nc.tensor.matmul(
                            zt[:, i, 0:rb, :].rearrange("k r t -> k (r t)"),
                            entH_sb[:, b * E:(b + 1) * E],
                            U_sb[:, r0:r0 + rb, b * E:(b + 1) * E],
                            start=True, stop=True,
                        )
                    if bg % 2 == 0 or (last and b0 + nb == B):
                        nc.vector.tensor_copy(
                            z_sb[:, b0:b0 + nb, 0:rb, :],
                            zt[:, 0:nb, 0:rb, :])
                    else:
                        nc.scalar.copy(
                            z_sb[:, b0:b0 + nb, 0:rb, :],
                            zt[:, 0:nb, 0:rb, :])
                    if last:
                        # final block: drain per b-group, engines round-robin,
                        # so the critical tail is one small DMA
                        dma_engs[bg % 3].dma_start(
                            out_z[:, b0:b0 + nb, r0:r0 + rb, :],
                            z_sb[:, b0:b0 + nb, 0:rb, :])
                if not last:
                    # whole block in one DMA, overlapped with later blocks
                    nc.gpsimd.dma_start(
                        out_z[:, :, r0:r0 + rb, :],
                        z_sb[:, :, 0:rb, :])
                r0 += rb

            # DCE keeper for the warm-up matmuls
            wsb = cpool.tile([1, 64], f32, tag="wsb")
            nc.vector.tensor_copy(wsb[:], wps[0:1, :])
            nc.sync.dma_start(dbg_d[:], wsb[:])
    nc.compile()
    return nc


def _run_device(in_maps):
    from concourse import bass_utils
    if "nc" not in _CACHE:
        _CACHE["nc"] = _build_bass()
    res = bass_utils.run_bass_kernel_spmd(
        _CACHE["nc"], in_maps, core_ids=list(range(NCORES)))
    return [np.asarray(r["out_z"], np.float32) for r in res.results]


def _postprocess(z_parts, R, bn1_gamma, bn1_beta, bn1_mean, bn1_var):
    z = z_parts[0]
    for p in z_parts[1:]:
        z = z + p                                # [k, b, r, t]
    z = z.transpose(1, 0, 3, 2)                  # [b, k, t, r]
    scale = bn1_gamma / np.sqrt(bn1_var + EPS)
    shift = bn1_beta - bn1_mean * scale
    Am = (R * scale[None, :]).T                  # [r, s]
    bias = R @ shift                             # [s]
    scores = z.reshape(B, E * E, D2) @ Am + bias
    return scores.reshape(B, E * E * R_NUM).astype(np.float32)


def kernel(encoder_hidden, entity_type, entity_id, mention_id,
           entity2mention_table, type_emb, id_emb, W, R,
           bn1_gamma, bn1_beta, bn1_mean, bn1_var):
    W = np.asarray(W, np.float32)
    in_maps, ent = _host_prepare(
        np.asarray(encoder_hidden, np.float32), np.asarray(entity_type),
        np.asarray(entity_id), np.asarray(mention_id),
        np.asarray(entity2mention_table, np.float32),
        np.asarray(type_emb, np.float32), np.asarray(id_emb, np.float32), W)
    try:
        z_parts = _run_device(in_maps)
    except Exception:  # fall back to exact host compute on any failure
        import traceback
        traceback.print_exc()
        ent_flat = ent.reshape(NBK, D)
        T = ent_flat @ W.reshape(D, D2 * D)                  # [192, 50*808]
        T = T.reshape(B, E, D2, D)
        z = np.einsum('bkrj,btj->bktr', T, ent)              # [b,k,t,r]
        scale = np.asarray(bn1_gamma) / np.sqrt(np.asarray(bn1_var) + EPS)
        zb = (z - np.asarray(bn1_mean)) * scale + np.asarray(bn1_beta)
        scores = zb.reshape(B, E * E, D2) @ np.asarray(R).T
        return scores.reshape(B, E * E * R_NUM).astype(np.float32)
    return _postprocess(z_parts, np.asarray(R, np.float32),
                        np.asarray(bn1_gamma, np.float32),
                        np.asarray(bn1_beta, np.float32),
                        np.asarray(bn1_mean, np.float32),
                        np.asarray(bn1_var, np.float32))



# revision 16
# speedup vs baseline: 1.3609x; 1.3609x over previous
"""Bass/Trainium2 kernel for nn_BERT_TUCKER (BERT + TuckER pair scoring).

z[b,k,t,r] = sum_{a,j} head[b,k,a] * Wv[a,r,j] * tail[b,t,j],
Wv = W.reshape(808, 50, 808)  (130.6 MB fp32; read-once => memory roofline).

Structure (per core, SPMD-uniform):
  The (a, r) column space of Wv (808*50 slots) is tiled into "units" of
  <=128 stationary columns: 300 big units (128 a's x 1 r, a-groups g0..g5)
  and 17 trio units (the 40-wide a-remainder x 3 r's = 120 cols).  Each
  core owns 41 unit slots: 3 r-blocks of 10 big + 1 r-block of 8 big
  (mono-a-group each) + 1 trio block of 3 (a few slots are zero pads so
  all 8 cores run an identical instruction stream).

  m1 (per unit): U[cols, bt] = sum_j W_unit[j, cols].T @ entT[j, bt],
     7 j-chunks of 128 (last 40), N=192 -> 41*7 matmuls of N=192.
  m2 (per r-block): stationary lhsT = U slots, moving rhs = pooled heads
     (12 cols per b) -> N=12 per matmul; trio block uses a host-built
     selector matrix (block-diagonal heads) with N=36.
  Host sums the per-core z partials and applies batchnorm + R (affine in
  z, so exact).

W is bf16 (8.5 MB/core); its DMA is split across all three DMA queues
(sync/scalar/gpsimd) so transfers overlap, and the whole W slice stays
resident in SBUF.  Mention/entity pooling runs on host into ent.
"""

import numpy as np
from ml_dtypes import bfloat16

B, S, H = 16, 512, 768
TS, IS = 20, 20
D = H + TS + IS          # 808
M = 36
E = 12
R_NUM = 97
D2 = 50
EPS = 1e-5

NCORES = 8
NBK = B * E              # 192 = (b, entity) pairs
NJC = 7                  # j chunks of 128 (last has 40 real rows)
JC6 = D - (NJC - 1) * 128    # 40
GW = 128                 # a-group width (g0..g5); g6 is 40 wide
NG = 6                   # full 128-wide a-groups
AR = D - NG * GW         # 40 = a-remainder width
NU = 40                  # unit slots per core
RB_SIZES = (8, 8, 8, 8, 5)   # big r-blocks (device m2)
RB_STARTS = (0, 8, 16, 24, 32)
SHIP_S0 = 37             # slots 37..39: U ships to host (host does their m2)
NWARM = 20               # PE ramp-keeper matmuls (N=64) before first W lands
NUDMA = 3                # trailing units whose U ships to host (tail cut)

_CACHE = {}


# ---------------------------------------------------------------- assignment
def _assignment():
    """Global (a-group, r) -> (core, slot) layout.  Returns per-core:
    rb8s: 4 x (g, r-list of 8); rb5: (g, r-list of 5); ships: 3 x
    (list of (a0, aw, r) segments | None)."""
    rb8 = [(g, list(range(8 * i, 8 * i + 8)))
           for g in range(4) for i in range(5)]                       # 20
    rb8 += [(g, list(range(8 * i, 8 * i + 8)))
            for g in (4, 5) for i in range(6)]                        # +12
    rb5 = [(g, list(range(40 + 5 * i, 45 + 5 * i)))
           for g in range(4) for i in range(2)]                       # 8
    ships = [[(NG * GW, AR, r) for r in range(3 * t, min(3 * t + 3, D2))]
             for t in range(17)]                                      # trios
    ships += [[(4 * GW, GW, 48)], [(4 * GW, GW, 49)],
              [(5 * GW, GW, 48)], [(5 * GW, GW, 49)]]                 # 21
    ships += [None] * 3                                               # 24
    cores = []
    for c in range(NCORES):
        cores.append((rb8[4 * c:4 * c + 4], rb5[c],
                      ships[3 * c:3 * c + 3]))
    return cores


ASSIGN = _assignment()


# ------------------------------------------------------------------- pooling
def _pool_entities(encoder_hidden, entity_type, entity_id, mention_id,
                   entity2mention_table, type_emb, id_emb):
    """Steps 1-3 of the reference (embedding concat + mention/entity
    pooling) on host. Returns ent [B, E, D] fp32."""
    enc = np.concatenate(
        [encoder_hidden, type_emb[entity_type], id_emb[entity_id]], axis=-1
    ).astype(np.float32)                                   # [B,S,D]
    cls = np.concatenate(
        [encoder_hidden[:, 0, :], np.zeros((B, TS + IS), np.float32)], axis=-1
    )                                                      # [B,D]

    sel = (np.arange(1, M + 1, dtype=mention_id.dtype)[None, :, None]
           == mention_id[:, None, :]).astype(np.float32)   # [B,M,S]
    cnt = sel.sum(axis=-1, keepdims=True)
    sel = np.where(cnt > 0, sel / np.maximum(cnt, 1), sel)
    x = np.matmul(sel, enc)                                # [B,M,D]
    x = np.concatenate([cls[:, None, :], x], axis=1)       # [B,M+1,D]

    tbl = entity2mention_table.astype(np.float32).copy()
    tbl[:, 0, 0] = 1.0
    mcnt = tbl.sum(axis=-1, keepdims=True)
    tbl = np.where(mcnt > 0, tbl / np.maximum(mcnt, 1), tbl)
    return np.matmul(tbl, x)[:, 1:, :]                     # [B,E,D]


# ---------------------------------------------------------------- W prepare
def _w_fingerprint(W):
    s = np.ascontiguousarray(W[::7, ::101, ::97])
    return (W.shape, str(W.dtype), s.tobytes())


def _unit_cols(core):
    """Per-slot column->(a, r) maps for one core.  Returns (A, R) int arrays
    [NU, 128]; -1 where the column is a zero pad."""
    rb8s, rb5, ships = core
    A = -np.ones((NU, 128), np.int64)
    R = np.zeros((NU, 128), np.int64)
    for i, (g, rl) in enumerate(list(rb8s) + [rb5]):
        for si, r in enumerate(rl):
            u = RB_STARTS[i] + si
            A[u] = np.arange(g * GW, (g + 1) * GW)
            R[u] = r
    for i, segs in enumerate(ships):
        if segs is None:
            continue
        u = SHIP_S0 + i
        m0 = 0
        for (a0, aw, r) in segs:
            A[u, m0:m0 + aw] = np.arange(a0, a0 + aw)
            R[u, m0:m0 + aw] = r
            m0 += aw
    return A, R


def _prepare_w(W):
    """Per-core W unit tensors: Wmain [128, NU, 6, 128] (j chunks 0-5) and
    Wtail [40, NU, 128] (j chunk 6), bf16.  Cached (W is static)."""
    key = _w_fingerprint(W)
    hit = _CACHE.get("wprep")
    if hit is not None and hit[0] == key:
        return hit[1]
    Wv = W.reshape(D, D2, D)                               # [a, r, j]
    out = []
    for core in ASSIGN:
        A, R = _unit_cols(core)
        Am = np.maximum(A, 0)
        Wslice = Wv[Am, R, :]                              # [NU, 128, 808]
        Wslice[A < 0] = 0.0
        wmain = np.ascontiguousarray(
            Wslice[:, :, :NG * GW].reshape(NU, 128, 6, 128)
            .transpose(3, 0, 2, 1)).astype(bfloat16)       # [128,NU,6,128]
        wtail = np.ascontiguousarray(
            Wslice[:, :, NG * GW:].transpose(2, 0, 1)).astype(bfloat16)
        out.append((wmain, wtail))                         # tail [40,NU,128]
    _CACHE["wprep"] = (key, out)
    return out


# ------------------------------------------------------------- host prepare
def _host_prepare(encoder_hidden, entity_type, entity_id, mention_id,
                  entity2mention_table, type_emb, id_emb, W):
    ent = _pool_entities(encoder_hidden, entity_type, entity_id, mention_id,
                         entity2mention_table, type_emb, id_emb)
    ent_flat = ent.reshape(NBK, D)                         # [(b,e), D]

    entT = np.zeros((NJC * 128, NBK), np.float32)
    entT[:D] = ent_flat.T
    entT_dev = np.ascontiguousarray(
        entT.astype(bfloat16).reshape(NJC, 128, NBK).transpose(1, 0, 2))

    w_cores = _prepare_w(W)
    in_maps = []
    for c, core in enumerate(ASSIGN):
        rb8s, rb5, _ = core
        gs = [g for g, _ in rb8s] + [rb5[0]]
        rh = np.stack([ent_flat[:, g * GW:(g + 1) * GW].T for g in gs],
                      axis=1)                              # [128, 5, 192]
        in_maps.append({
            "Wmain": w_cores[c][0],
            "Wtail": w_cores[c][1],
            "entT": entT_dev,
            "RH": np.ascontiguousarray(rh.astype(bfloat16)),
        })
    return in_maps, ent


# ------------------------------------------------------------------- device
def _build_bass():
    import concourse.bacc as bacc
    import concourse.mybir as mybir
    import concourse.tile as tile

    f32 = mybir.dt.float32
    bf16 = mybir.dt.bfloat16

    nc = bacc.Bacc("TRN2", target_bir_lowering=False, debug=False)
    Wmain_d = nc.dram_tensor("Wmain", (128, NU, 6, 128), bf16,
                             kind="ExternalInput")
    Wtail_d = nc.dram_tensor("Wtail", (JC6, NU, 128), bf16,
                             kind="ExternalInput")
    entT_d = nc.dram_tensor("entT", (128, NJC, NBK), bf16,
                            kind="ExternalInput")
    RH_d = nc.dram_tensor("RH", (128, 5, NBK), bf16, kind="ExternalInput")
    out_z = nc.dram_tensor("out_z", (96, 5, NBK), f32, kind="ExternalOutput")
    out_U = nc.dram_tensor("out_U", (128, NUDMA, NBK), bf16,
                           kind="ExternalOutput")
    dbg_d = nc.dram_tensor("dbg", (1, 64), f32, kind="ExternalOutput")

    # W DMA blocks: (queue, u0, u1); queues overlap, per-queue serial.
    # Interleaved across the 3 queues so delivery runs ahead of the PE's
    # ~0.56us/unit consumption from the start.
    blocks = [("sync", 0, 1), ("gpsimd", 1, 2), ("scalar", 2, 4),
              ("sync", 4, 6), ("scalar", 6, 10), ("sync", 10, 14),
              ("gpsimd", 14, 18), ("scalar", 18, 22), ("sync", 22, 26),
              ("gpsimd", 26, 30), ("scalar", 30, 34), ("sync", 34, 37),
              ("gpsimd", 37, 40)]

    with tile.TileContext(nc) as tc:
        with (
            tc.tile_pool(name="const", bufs=1) as cpool,
            tc.tile_pool(name="ps_u", bufs=5, space="PSUM") as ps_u,
            tc.tile_pool(name="ps_z", bufs=3, space="PSUM") as ps_z,
        ):
            entT_sb = cpool.tile([128, NJC, NBK], bf16, tag="entT")
            nc.sync.dma_start(
                entT_sb[:, 0:2, :].rearrange("p a b -> p (a b)"),
                entT_d[:, 0:2, :].rearrange("p a b -> p (a b)"))
            nc.scalar.dma_start(
                entT_sb[:, 2:NJC, :].rearrange("p a b -> p (a b)"),
                entT_d[:, 2:NJC, :].rearrange("p a b -> p (a b)"))
            RH_sb = cpool.tile([128, 5, NBK], bf16, tag="RH")

            wm_sb = cpool.tile([128, NU, 6, 128], bf16, tag="wm")
            wt_sb = cpool.tile([JC6, NU, 128], bf16, tag="wt")
            for bi, (eng, u0, u1) in enumerate(blocks):
                q = getattr(nc, eng)
                q.dma_start(
                    wm_sb[:, u0:u1, :, :].rearrange("p u c a -> p (u c a)"),
                    Wmain_d[:, u0:u1, :, :].rearrange("p u c a -> p (u c a)"))
                q.dma_start(
                    wt_sb[:, u0:u1, :].rearrange("p u a -> p (u a)"),
                    Wtail_d[:, u0:u1, :].rearrange("p u a -> p (u a)"))
                if bi == 4:  # RH after scalar's second W block
                    nc.scalar.dma_start(
                        RH_sb[:].rearrange("p a b -> p (a b)"),
                        RH_d[:].rearrange("p a b -> p (a b)"))

            U_sb = cpool.tile([128, NU, NBK], bf16, tag="U")
            z_sb = cpool.tile([96, 5, NBK], f32, tag="z")

            # PE ramp keeper: stay busy until the first W block + entT land
            # (~2.8us: hwdge + transfer + sem prop).  Idle resets the ramp.
            wu = cpool.tile([128, 64], bf16, tag="warm")
            nc.vector.memset(wu[:, 0:1], 0.0)
            wps = ps_z.tile([1, 64], f32, tag="zt")
            for i in range(NWARM):
                nc.tensor.matmul(wps[:], wu[:, 0:1], wu[:],
                                 start=(i == 0), stop=(i == NWARM - 1))

            def m2_big(i, zq, sn=None):
                s0 = RB_STARTS[i]
                sn = RB_SIZES[i] if sn is None else sn
                zt = ps_z.tile([sn * E, B, E], f32, tag="zt")
                for b in range(B):
                    nc.tensor.matmul(
                        zt[:, b, :],
                        U_sb[:, s0:s0 + sn, b * E:(b + 1) * E],
                        RH_sb[:, i, b * E:(b + 1) * E],
                        start=True, stop=True)
                nc.vector.tensor_copy(z_sb[0:sn * E, i, :], zt[:])
                zq.dma_start(out_z[0:sn * E, i, :], z_sb[0:sn * E, i, :])

            for u in range(NU):
                pu = ps_u.tile([128, NBK], f32, tag="pu")
                for jc in range(6):
                    nc.tensor.matmul(pu[:], wm_sb[:, u, jc, :],
                                     entT_sb[:, jc, :],
                                     start=(jc == 0), stop=False)
                nc.tensor.matmul(pu[:], wt_sb[:, u, :],
                                 entT_sb[0:JC6, 6, :],
                                 start=False, stop=True)
                nc.vector.tensor_copy(U_sb[:, u, :], pu[:])
                if u >= NU - NUDMA:
                    # trailing units: ship U to host (host does their m2);
                    # avoids the copy->m2->copy->DMA chain on the tail
                    uq = (nc.gpsimd, nc.scalar, nc.sync)[u - (NU - NUDMA)]
                    uq.dma_start(out_U[:, u - (NU - NUDMA), :], U_sb[:, u, :])
                # emit m2 for a finished r-block a few units later (copy lag)
                if u == 10:
                    m2_big(0, nc.gpsimd)    # slots 0-7
                elif u == 18:
                    m2_big(1, nc.sync)      # slots 8-15
                elif u == 26:
                    m2_big(2, nc.gpsimd)    # slots 16-23
                elif u == 34:
                    m2_big(3, nc.scalar)    # slots 24-31
                elif u == NU - 1:
                    m2_big(4, nc.sync)      # slots 32-36 (rblk5)


            wsb = cpool.tile([1, 64], f32, tag="wsb")
            nc.vector.tensor_copy(wsb[:], wps[0:1, :])
            nc.sync.dma_start(dbg_d[:], wsb[:])
    nc.compile()
    return nc


def _run_device(in_maps):
    from concourse import bass_utils
    if "nc" not in _CACHE:
        _CACHE["nc"] = _build_bass()
    res = bass_utils.run_bass_kernel_spmd(
        _CACHE["nc"], in_maps, core_ids=list(range(NCORES)))
    return [{"out_z": np.asarray(r["out_z"], np.float32),
             "out_U": np.asarray(r["out_U"], np.float32)}
            for r in res.results]


# ------------------------------------------------------------------- gather
def _gather_z(parts, ent_flat):
    z = np.zeros((B, E, D2, E), np.float32)                # [b, k, r, t]
    entB = ent_flat.reshape(B, E, D)                       # [b, k, a]
    for c, (rb8s, rb5, ships) in enumerate(ASSIGN):
        P = parts[c]["out_z"].reshape(8, E, 5, B, E)       # [si,t,blk,b,k]
        for i, (g, rl) in enumerate(list(rb8s) + [rb5]):
            blk = P[:len(rl), :, i]                        # [si, t, b, k]
            z[:, :, rl, :] += blk.transpose(2, 3, 0, 1)
        # shipped slots: host-side m2 from U
        U3 = parts[c]["out_U"].reshape(GW, NUDMA, B, E)    # [p, i, b, t]
        for idx, segs in enumerate(ships):
            if segs is None:
                continue
            m0 = 0
            for (a0, aw, r) in segs:
                H = entB[:, :, a0:a0 + aw]                 # [b, k, aw]
                z[:, :, r, :] += np.einsum(
                    'pbt,bkp->bkt', U3[m0:m0 + aw, idx], H)
                m0 += aw
    return z


def _postprocess(z, R, bn1_gamma, bn1_beta, bn1_mean, bn1_var):
    scale = bn1_gamma / np.sqrt(bn1_var + EPS)
    shift = bn1_beta - bn1_mean * scale
    Am = (R * scale[None, :]).T                  # [r, s]
    bias = R @ shift                             # [s]
    zp = z.transpose(0, 1, 3, 2).reshape(B, E * E, D2)   # [b, (k,t), r]
    scores = zp @ Am + bias
    return scores.reshape(B, E * E * R_NUM).astype(np.float32)


def kernel(encoder_hidden, entity_type, entity_id, mention_id,
           entity2mention_table, type_emb, id_emb, W, R,
           bn1_gamma, bn1_beta, bn1_mean, bn1_var):
    W = np.asarray(W, np.float32)
    in_maps, ent = _host_prepare(
        np.asarray(encoder_hidden, np.float32), np.asarray(entity_type),
        np.asarray(entity_id), np.asarray(mention_id),
        np.asarray(entity2mention_table, np.float32),
        np.asarray(type_emb, np.float32), np.asarray(id_emb, np.float32), W)
    try:
        parts = _run_device(in_maps)
        z = _gather_z(parts, ent.reshape(NBK, D))
    except Exception:  # fall back to exact host compute on any failure
        import traceback
        traceback.print_exc()
        ent_flat = ent.reshape(NBK, D)
        T = ent_flat @ W.reshape(D, D2 * D)                  # [192, 50*808]
        T = T.reshape(B, E, D2, D)
        z = np.einsum('bkrj,btj->bkrt', T, ent)              # [b,k,r,t]
    return _postprocess(z, np.asarray(R, np.float32),
                        np.asarray(bn1_gamma, np.float32),
                        np.asarray(bn1_beta, np.float32),
                        np.asarray(bn1_mean, np.float32),
                        np.asarray(bn1_var, np.float32))
